# revision 27
# baseline (speedup 1.0000x reference)
"""Trainium2 Bass kernel for nn_AttentionBlock (GroupNorm + MHA + proj + residual).

Full inputs in, full output out. Sharding: 8 cores = 2 batches x 4 query-slices.
Each core: GroupNorm over its batch image (replicated within the batch group),
q projection for its 1024 queries, k/v projections over all 4096 keys,
per-head attention (S^T = k^T q formulation, softmax along the PSUM partition
axis via an appended ones-column in the PV matmul), output projection and
residual for its query slice. Host side only slices/rotates/concatenates.

All matmuls run in bf16 with fp32 PSUM accumulation; softmax logits stay fp32.
"""
import numpy as np

C = 512          # channels
N = 4096         # pixels (64*64)
NQ = 1024        # queries per core
H = 8            # heads
D = 64           # head dim
T = 4            # 128-channel chunks
W = NQ // 512    # query windows of 512
MT = N // 128    # key m-tiles of 128
NGROUPS = 8
EPS = 1e-5
GELEM = (C // NGROUPS) * N   # elements per norm group
MG = [3] * 10 + [2]          # m-tile group sizes (PSUM bank batching for exp)

DEBUG = False                # adds intermediate-dump DRAM outputs

_COMPILED = None


def _emit(tc, io):
    import concourse.bass as bass
    from concourse import mybir, bass_isa
    from contextlib import ExitStack

    nc = tc.nc
    f32 = mybir.dt.float32
    bf16 = mybir.dt.bfloat16
    Alu = mybir.AluOpType
    Act = mybir.ActivationFunctionType

    xb, qkvw, qkvb, projw, projb, nw, nb, y = (
        io["xb"], io["qkvw"], io["qkvb"], io["projw"], io["projb"],
        io["nw"], io["nb"], io["y"])

    ctx = ExitStack()
    with ctx:
        # ---------------- pools ----------------
        # PSUM: big pool 2x(128,1536) [6 banks] shared by S-tiles, phase-3
        # accumulators and weight transposes; pv gets its own bank; bc/proj
        # share one more. 6+1+1 = 8 banks.
        left = ctx.enter_context(tc.tile_pool(name="left", bufs=1))
        psum_big = ctx.enter_context(tc.tile_pool(name="psum_big", bufs=2, space="PSUM"))
        psum_pv = ctx.enter_context(tc.tile_pool(name="psum_pv", bufs=1, space="PSUM"))
        psum_acc = ctx.enter_context(tc.tile_pool(name="psum_acc", bufs=1, space="PSUM"))

        right_ctx = ExitStack()
        xf_pool = right_ctx.enter_context(
            tc.tile_pool(name="xf_pool", bufs=1, side="right"))
        wstg_pool = right_ctx.enter_context(
            tc.tile_pool(name="wstg_pool", bufs=4, side="right"))
        scr_pool = right_ctx.enter_context(
            tc.tile_pool(name="scr_pool", bufs=2, side="right"))

        # ---------------- persistent tiles ----------------
        xn = [left.tile([128, N], bf16, name=f"xn{t}", tag=f"xn{t}") for t in range(T)]
        ksb = [left.tile([128, N], bf16, name=f"ksb{t}", tag=f"ksb{t}") for t in range(T)]
        qsb = [left.tile([128, NQ], bf16, name=f"qsb{t}", tag=f"qsb{t}") for t in range(T)]
        wTq = [left.tile([128, 1536], bf16, name=f"wTq{t}", tag=f"wTq{t}") for t in range(T)]
        wTp = [left.tile([128, C], bf16, name=f"wTp{t}", tag=f"wTp{t}") for t in range(T)]
        vb_bc = left.tile([128, C], f32, name="vb_bc", tag="vb_bc")
        ones_row = left.tile([1, D], f32, name="ones_row", tag="ones_row")
        qb = [left.tile([128, 1], f32, name=f"qb{i}", tag=f"qb{i}") for i in range(8)]
        pb = [left.tile([128, 1], f32, name=f"pb{i}", tag=f"pb{i}") for i in range(T)]
        nwt = [left.tile([128, 1], f32, name=f"nwt{t}", tag=f"nwt{t}") for t in range(T)]
        nbt = [left.tile([128, 1], f32, name=f"nbt{t}", tag=f"nbt{t}") for t in range(T)]
        stat = [left.tile([128, 2], f32, name=f"stat{t}", tag=f"stat{t}") for t in range(T)]
        gstat = [left.tile([128, 2], f32, name=f"gstat{t}", tag=f"gstat{t}") for t in range(T)]

        # ---------------- input DMAs ----------------
        xf = [xf_pool.tile([128, N], f32, name=f"xf{t}", tag=f"xf{t}") for t in range(T)]
        for t in range(T):
            for c4 in range(4):   # split across DMA queues
                nc.sync.dma_start(
                    xf[t][:, 1024 * c4:1024 * (c4 + 1)],
                    xb[128 * t:128 * (t + 1), 1024 * c4:1024 * (c4 + 1)])
            nc.sync.dma_start(nwt[t][:, 0:1], nw[128 * t:128 * (t + 1)])
            nc.sync.dma_start(nbt[t][:, 0:1], nb[128 * t:128 * (t + 1)])
            nc.sync.dma_start(pb[t][:, 0:1], projb[128 * t:128 * (t + 1)])
        for i in range(8):
            nc.sync.dma_start(qb[i][:, 0:1], qkvb[128 * i:128 * (i + 1)])
        # v bias broadcast to 128 partitions (stride-0 partition read)
        nc.gpsimd.dma_start(
            out=vb_bc[:],
            in_=bass.AP(tensor=qkvb.tensor, offset=1024, ap=[[0, 128], [1, C]]))
        nc.vector.memset(ones_row[0:1, :], 1.0)

        # weights: natural-layout contiguous DMA, cast to bf16, transpose
        # 128x128 blocks on the PE (identity trick) into wTq/wTp.
        # identity + group-indicator matrices come in as constant inputs
        # (gpsimd ucode for iota/affine_select is unavailable here)
        ident = left.tile([128, 128], bf16, name="ident", tag="ident")
        nc.sync.dma_start(ident[:], io["cid"][:, :])
        ind = left.tile([128, 2], f32, name="ind", tag="ind")
        nc.sync.dma_start(ind[:], io["cind"][:, :])
        indT = left.tile([2, 128], f32, name="indT", tag="indT")
        nc.sync.dma_start(indT[0:2, :], io["cindT"][:, :])
        for i in range(12):   # qkv_w row-tiles
            wstg = wstg_pool.tile([128, C], f32, name="wstg", tag="wstg")
            nc.sync.dma_start(wstg[:], qkvw[128 * i:128 * (i + 1), :])
            wbf = wstg_pool.tile([128, C], bf16, name="wbf", tag="wbf")
            nc.vector.tensor_copy(wbf[:], wstg[:])
            for j in range(T):
                tp = psum_big.tile([128, 128], bf16, name="tp", tag="sbig")
                nc.tensor.transpose(tp[:], wbf[:, 128 * j:128 * (j + 1)], ident[:])
                nc.vector.tensor_copy(wTq[j][:, 128 * i:128 * (i + 1)], tp[:])
        for i in range(4):    # proj_w row-tiles
            wstg = wstg_pool.tile([128, C], f32, name="wstg", tag="wstg")
            nc.sync.dma_start(wstg[:], projw[128 * i:128 * (i + 1), :])
            wbf = wstg_pool.tile([128, C], bf16, name="wbf", tag="wbf")
            nc.vector.tensor_copy(wbf[:], wstg[:])
            for j in range(T):
                tp = psum_big.tile([128, 128], bf16, name="tp", tag="sbig")
                nc.tensor.transpose(tp[:], wbf[:, 128 * j:128 * (j + 1)], ident[:])
                nc.vector.tensor_copy(wTp[j][:, 128 * i:128 * (i + 1)], tp[:])

        # ---------------- phase 1: group stats ----------------
        for t in range(T):
            nc.vector.tensor_reduce(
                out=stat[t][:, 0:1], in_=xf[t][:], axis=mybir.AxisListType.X, op=Alu.add)
            sq_scr = scr_pool.tile([128, N], bf16, name="sq_scr", tag="sq_scr")
            nc.scalar.activation(
                sq_scr[:], xf[t][:], Act.Square, accum_out=stat[t][:, 1:2])
            # group-reduce over partitions via indicator matmuls:
            # gg[g,s] = sum_ch ind[ch,g]*stat[ch,s]; then broadcast back
            # per channel: gstat[ch,s] = sum_g indT[g,ch]*gg[g,s]
            gg_ps = psum_acc.tile([2, 2], f32, name="gg_ps", tag="acc")
            nc.tensor.matmul(gg_ps[0:2, :], ind[:, 0:2], stat[t][:, 0:2],
                             start=True, stop=True)
            gg_sb = left.tile([2, 2], f32, name=f"gg_sb{t}", tag=f"gg_sb{t}")
            nc.vector.tensor_copy(gg_sb[0:2, :], gg_ps[0:2, :])
            gb_ps = psum_acc.tile([128, 2], f32, name="gb_ps", tag="acc")
            nc.tensor.matmul(gb_ps[:, 0:2], indT[0:2, :], gg_sb[0:2, :],
                             start=True, stop=True)
            nc.vector.tensor_copy(gstat[t][:, 0:2], gb_ps[:, 0:2])
            # mean/var/rstd -> per-channel affine a,b
            mean_t = left.tile([128, 1], f32, name=f"mean{t}", tag=f"mean{t}")
            e2_t = left.tile([128, 1], f32, name=f"e2{t}", tag=f"e2{t}")
            var_t = left.tile([128, 1], f32, name=f"var{t}", tag=f"var{t}")
            std_t = left.tile([128, 1], f32, name=f"std{t}", tag=f"std{t}")
            a_t = left.tile([128, 1], f32, name=f"a{t}", tag=f"a{t}")
            b_t = left.tile([128, 1], f32, name=f"b{t}", tag=f"b{t}")
            inv = 1.0 / GELEM
            nc.vector.tensor_scalar(mean_t[:], gstat[t][:, 0:1], inv, None, Alu.mult)
            nc.vector.tensor_scalar(e2_t[:], gstat[t][:, 1:2], inv, None, Alu.mult)
            nc.vector.scalar_tensor_tensor(
                var_t[:], mean_t[:], -1.0, mean_t[:], Alu.mult, Alu.mult)
            nc.vector.scalar_tensor_tensor(
                var_t[:], e2_t[:], EPS, var_t[:], Alu.add, Alu.add)
            nc.scalar.activation(std_t[:], var_t[:], Act.Sqrt)
            nc.vector.reciprocal(a_t[:], std_t[:])
            nc.vector.tensor_tensor(a_t[:], a_t[:], nwt[t][:], Alu.mult)
            nc.vector.tensor_tensor(b_t[:], mean_t[:], a_t[:], Alu.mult)
            nc.vector.tensor_tensor(b_t[:], nbt[t][:], b_t[:], Alu.subtract)
            # phase 2: normalize + cast
            nc.vector.tensor_scalar(
                xn[t][:], xf[t][:], a_t[:, 0:1], b_t[:, 0:1], Alu.mult, Alu.add)
            if DEBUG:
                d = io["dbg_ab"]
                nc.sync.dma_start(d[128 * t:128 * t + 128, 0:1], a_t[:])
                nc.sync.dma_start(d[128 * t:128 * t + 128, 1:2], b_t[:])
                nc.sync.dma_start(d[128 * t:128 * t + 128, 2:3], stat[t][:, 0:1])
                nc.sync.dma_start(d[128 * t:128 * t + 128, 3:4], stat[t][:, 1:2])
                nc.sync.dma_start(d[128 * t:128 * t + 128, 4:5], gstat[t][:, 0:1])
                nc.sync.dma_start(d[128 * t:128 * t + 128, 5:6], gstat[t][:, 1:2])
                nc.sync.dma_start(io["dbg_xn"][128 * t:128 * t + 128, :], xn[t][:, 0:64])

        right_ctx.close()

        # ---------------- mid pools (reuse xf space) ----------------
        mid = ctx.enter_context(tc.tile_pool(name="mid", bufs=1))
        psb_pool = ctx.enter_context(tc.tile_pool(name="psb_pool", bufs=3))
        rec_pool = ctx.enter_context(tc.tile_pool(name="rec_pool", bufs=2))
        yo_pool = ctx.enter_context(tc.tile_pool(name="yo_pool", bufs=2))

        vT = mid.tile([128, MT * 520], bf16, name="vT", tag="vT")
        attn = [mid.tile([128, NQ], bf16, name=f"attn{t}", tag=f"attn{t}") for t in range(T)]
        xres = [mid.tile([128, NQ], f32, name=f"xres{t}", tag=f"xres{t}") for t in range(T)]
        for t in range(T):
            nc.sync.dma_start(xres[t][:], xb[128 * t:128 * (t + 1), 0:NQ])

        # ones columns of the augmented v^T (denominator trick)
        ones_view = vT[:].rearrange("p (m h x) -> p m h x", m=MT, x=65)[:, :, :, 64:65]
        nc.vector.memset(ones_view, 1.0)

        # ---------------- phase 3: projections ----------------
        # q: out rows 0..511 of qkv, only NQ query columns
        for i in range(T):
            for w in range(W):
                qp = psum_big.tile([128, 512], f32, name="qp", tag="sbig")
                for k in range(T):
                    nc.tensor.matmul(
                        qp[:], wTq[k][:, 128 * i:128 * i + 128],
                        xn[k][:, 512 * w:512 * w + 512],
                        start=(k == 0), stop=(k == T - 1))
                nc.vector.tensor_scalar(
                    qsb[i][:, 512 * w:512 * w + 512], qp[:], qb[i][:, 0:1], None, Alu.add)
        # k: out rows 512..1023, all N columns
        for i in range(T):
            for w in range(N // 512):
                kp = psum_big.tile([128, 512], f32, name="kp", tag="sbig")
                for k in range(T):
                    nc.tensor.matmul(
                        kp[:], wTq[k][:, 512 + 128 * i:512 + 128 * i + 128],
                        xn[k][:, 512 * w:512 * w + 512],
                        start=(k == 0), stop=(k == T - 1))
                nc.vector.tensor_scalar(
                    ksb[i][:, 512 * w:512 * w + 512], kp[:], qb[4 + i][:, 0:1], None, Alu.add)
        # vT: (m, 512) per m-tile, strided into the 65-column augmented layout
        for mt in range(MT):
            vp = psum_big.tile([128, 512], f32, name="vp", tag="sbig")
            for k in range(T):
                nc.tensor.matmul(
                    vp[:], xn[k][:, 128 * mt:128 * mt + 128],
                    wTq[k][:, 1024:1536],
                    start=(k == 0), stop=(k == T - 1))
            dst = vT[:, 520 * mt:520 * mt + 520].rearrange(
                "p (h x) -> p h x", x=65)[:, :, 0:64]
            src = vp[:].rearrange("p (h x) -> p h x", x=64)
            vbv = vb_bc[:].rearrange("p (h x) -> p h x", x=64)
            nc.vector.tensor_tensor(dst, src, vbv, Alu.add)
            if DEBUG and mt == 0:
                nc.sync.dma_start(io["dbg_vt"][:], vT[:, 0:520])
                nc.sync.dma_start(io["dbg_k"][0:128, :], ksb[0][:, 0:64])
                nc.sync.dma_start(io["dbg_q"][0:128, :], qsb[0][:, 0:64])

        # ---------------- phase 4: attention ----------------
        for w in range(W):
            for h in range(H):
                kt, pr = h // 2, 64 * (h % 2)
                pv = psum_pv.tile([128, 512], f32, name="pv", tag="pv")
                mt = 0
                for gs in MG:
                    sp = psum_big.tile([128, 1536], f32, name="sp", tag="sbig")
                    for j in range(gs):
                        nc.tensor.matmul(
                            sp[:, 512 * j:512 * j + 512],
                            ksb[kt][pr:pr + 64, 128 * (mt + j):128 * (mt + j) + 128],
                            qsb[kt][pr:pr + 64, 512 * w:512 * w + 512],
                            start=True, stop=True)
                    ps = psb_pool.tile([128, 1536], bf16, name="ps", tag="ps")
                    nc.scalar.activation(
                        ps[:, 0:512 * gs], sp[:, 0:512 * gs], Act.Exp, scale=0.125)
                    for j in range(gs):
                        m = mt + j
                        nc.tensor.matmul(
                            pv[0:65, :],
                            vT[:, 520 * m + 65 * h:520 * m + 65 * h + 65],
                            ps[:, 512 * j:512 * j + 512],
                            start=(m == 0), stop=(m == MT - 1))
                    mt += gs
                rec = rec_pool.tile([1, 512], f32, name="rec", tag="rec")
                if DEBUG and w == 0:
                    dd = rec_pool.tile([1, 512], f32, name="dd", tag="dd")
                    dd2 = rec_pool.tile([1, 512], f32, name="dd2", tag="dd2")
                    nc.vector.tensor_copy(dd[0:1, :], pv[64:65, :])
                    nc.vector.tensor_copy(dd2[0:1, :], pv[0:1, :])
                    nc.sync.dma_start(io["dbg_den"][h:h + 1, :], dd[0:1, :])
                    nc.sync.dma_start(io["dbg_pv"][h:h + 1, :], dd2[0:1, :])
                # NOTE: reciprocal_approx_* mis-handles nonzero partition
                # offsets on HW (reads partition 0); exact reciprocal is safe
                nc.vector.reciprocal(rec[0:1, :], pv[64:65, :])
                bc = psum_acc.tile([128, 512], f32, name="bc", tag="acc")
                nc.tensor.matmul(
                    bc[0:64, :], ones_row[0:1, 0:D], rec[0:1, :], start=True, stop=True)
                bcs = rec_pool.tile([64, 512], f32, name="bcs", tag="bcs")
                nc.vector.tensor_copy(bcs[0:64, :], bc[0:64, :])
                nc.vector.tensor_tensor(
                    attn[kt][pr:pr + 64, 512 * w:512 * w + 512],
                    pv[0:64, :], bcs[0:64, :], Alu.mult)

            # ---------------- phase 5: proj + residual for this window ----
            for i in range(T):
                py = psum_acc.tile([128, 512], f32, name="py", tag="acc")
                # shares the 1-bank acc pool with bc; proj overlaps attention
                # of the next window only through this slot
                for k in range(T):
                    nc.tensor.matmul(
                        py[:], wTp[k][:, 128 * i:128 * i + 128],
                        attn[k][:, 512 * w:512 * w + 512],
                        start=(k == 0), stop=(k == T - 1))
                yo = yo_pool.tile([128, 512], f32, name="yo", tag="yo")
                nc.vector.scalar_tensor_tensor(
                    yo[:], py[:], pb[i][:, 0:1], xres[i][:, 512 * w:512 * w + 512],
                    Alu.add, Alu.add)
                nc.sync.dma_start(y[128 * i:128 * i + 128, 512 * w:512 * w + 512], yo[:])


def _build():
    import concourse.tile as tile
    from concourse import bacc, mybir

    nc = bacc.Bacc("TRN2", target_bir_lowering=False, debug=False)
    f32 = mybir.dt.float32
    io = {
        "xb": nc.dram_tensor("xb", [C, N], f32, kind="ExternalInput").ap(),
        "qkvw": nc.dram_tensor("qkvw", [3 * C, C], f32, kind="ExternalInput").ap(),
        "qkvb": nc.dram_tensor("qkvb", [3 * C], f32, kind="ExternalInput").ap(),
        "projw": nc.dram_tensor("projw", [C, C], f32, kind="ExternalInput").ap(),
        "projb": nc.dram_tensor("projb", [C], f32, kind="ExternalInput").ap(),
        "nw": nc.dram_tensor("nw", [C], f32, kind="ExternalInput").ap(),
        "nb": nc.dram_tensor("nb", [C], f32, kind="ExternalInput").ap(),
        "cid": nc.dram_tensor("cid", [128, 128], mybir.dt.bfloat16,
                              kind="ExternalInput").ap(),
        "cind": nc.dram_tensor("cind", [128, 2], f32, kind="ExternalInput").ap(),
        "cindT": nc.dram_tensor("cindT", [2, 128], f32, kind="ExternalInput").ap(),
        "y": nc.dram_tensor("y", [C, NQ], f32, kind="ExternalOutput").ap(),
    }
    if DEBUG:
        bf16 = mybir.dt.bfloat16
        io["dbg_ab"] = nc.dram_tensor("dbg_ab", [C, 8], f32, kind="ExternalOutput").ap()
        io["dbg_xn"] = nc.dram_tensor("dbg_xn", [C, 64], bf16, kind="ExternalOutput").ap()
        io["dbg_vt"] = nc.dram_tensor("dbg_vt", [128, 520], bf16, kind="ExternalOutput").ap()
        io["dbg_k"] = nc.dram_tensor("dbg_k", [C, 64], bf16, kind="ExternalOutput").ap()
        io["dbg_q"] = nc.dram_tensor("dbg_q", [C, 64], bf16, kind="ExternalOutput").ap()
        io["dbg_den"] = nc.dram_tensor("dbg_den", [8, 512], f32, kind="ExternalOutput").ap()
        io["dbg_pv"] = nc.dram_tensor("dbg_pv", [8, 512], f32, kind="ExternalOutput").ap()
    with tile.TileContext(nc) as tc:
        _emit(tc, io)
    nc.compile()
    return nc


def get_compiled():
    global _COMPILED
    if _COMPILED is None:
        _COMPILED = _build()
    return _COMPILED


def make_in_maps(x, norm_w, norm_b, qkv_w, qkv_b, proj_w, proj_b):
    import ml_dtypes

    xf = np.ascontiguousarray(np.asarray(x, np.float32)).reshape(2, C, N)
    ind = np.zeros((128, 2), np.float32)
    ind[0:64, 0] = 1.0
    ind[64:128, 1] = 1.0
    shared = {
        "cid": np.eye(128, dtype=ml_dtypes.bfloat16),
        "cind": ind,
        "cindT": np.ascontiguousarray(ind.T),
        "qkvw": np.ascontiguousarray(np.asarray(qkv_w, np.float32)),
        "qkvb": np.ascontiguousarray(np.asarray(qkv_b, np.float32)),
        "projw": np.ascontiguousarray(np.asarray(proj_w, np.float32)),
        "projb": np.ascontiguousarray(np.asarray(proj_b, np.float32)),
        "nw": np.ascontiguousarray(np.asarray(norm_w, np.float32)),
        "nb": np.ascontiguousarray(np.asarray(norm_b, np.float32)),
    }
    in_maps = []
    for core in range(8):
        bi, qs = core // 4, core % 4
        # rotate so this core's queries are always columns [0:NQ)
        xroll = np.concatenate(
            [xf[bi][:, qs * NQ:], xf[bi][:, :qs * NQ]], axis=1)
        m = dict(shared)
        m["xb"] = np.ascontiguousarray(xroll)
        in_maps.append(m)
    return in_maps


def assemble(results, x):
    y = np.zeros((2, C, N), np.float32)
    for core in range(8):
        bi, qs = core // 4, core % 4
        y[bi][:, qs * NQ:(qs + 1) * NQ] = results[core]["y"]
    return y.reshape(x.shape)


def kernel(x, norm_w, norm_b, qkv_w, qkv_b, proj_w, proj_b, **_ignored):
    from concourse import bass_utils

    nc = get_compiled()
    in_maps = make_in_maps(x, norm_w, norm_b, qkv_w, qkv_b, proj_w, proj_b)
    res = bass_utils.run_bass_kernel_spmd(nc, in_maps, core_ids=list(range(8)))
    return assemble(res.results, np.asarray(x))


# revision 32
# speedup vs baseline: 1.3270x; 1.3270x over previous
"""Trainium2 Bass kernel for nn_AttentionBlock (GroupNorm + MHA + proj + residual).

Full inputs in, full output out. Sharding: 8 cores = 2 batches x 4 query-slices.
Each core: GroupNorm over its batch image (replicated within the batch group),
q projection for its 1024 queries, k/v projections over all 4096 keys,
per-head attention (S^T = k^T q formulation, softmax along the PSUM partition
axis via an appended ones-column in the PV matmul), output projection and
residual for its query slice. Host side only slices/rotates/concatenates.

All matmuls run in bf16 with fp32 PSUM accumulation; softmax logits stay fp32.
"""
import numpy as np

C = 512          # channels
N = 4096         # pixels (64*64)
NQ = 1024        # queries per core
H = 8            # heads
D = 64           # head dim
T = 4            # 128-channel chunks
W = NQ // 512    # query windows of 512
MT = N // 128    # key m-tiles of 128
NGROUPS = 8
EPS = 1e-5
GELEM = (C // NGROUPS) * N   # elements per norm group
MG = [3] * 10 + [2]          # m-tile group sizes (PSUM bank batching for exp)

DEBUG = False                # adds intermediate-dump DRAM outputs

_COMPILED = None


def _emit(tc, io):
    import concourse.bass as bass
    from concourse import mybir, bass_isa
    from contextlib import ExitStack

    nc = tc.nc
    f32 = mybir.dt.float32
    bf16 = mybir.dt.bfloat16
    Alu = mybir.AluOpType
    Act = mybir.ActivationFunctionType

    xb, qkvw, qkvb, projw, projb, nw, nb, y = (
        io["xb"], io["qkvw"], io["qkvb"], io["projw"], io["projb"],
        io["nw"], io["nb"], io["y"])

    ctx = ExitStack()
    with ctx:
        # ---------------- pools ----------------
        # PSUM: big pool 2x(128,1536) [6 banks] shared by S-tiles, phase-3
        # accumulators and weight transposes; pv gets its own bank; bc/proj
        # share one more. 6+1+1 = 8 banks.
        left = ctx.enter_context(tc.tile_pool(name="left", bufs=1))
        psum_big = ctx.enter_context(tc.tile_pool(name="psum_big", bufs=2, space="PSUM"))
        psum_pv = ctx.enter_context(tc.tile_pool(name="psum_pv", bufs=1, space="PSUM"))
        psum_acc = ctx.enter_context(tc.tile_pool(name="psum_acc", bufs=1, space="PSUM"))

        right_ctx = ExitStack()
        xf_pool = right_ctx.enter_context(
            tc.tile_pool(name="xf_pool", bufs=1, side="right"))
        wstg_pool = right_ctx.enter_context(
            tc.tile_pool(name="wstg_pool", bufs=4, side="right"))
        scr_pool = right_ctx.enter_context(
            tc.tile_pool(name="scr_pool", bufs=2, side="right"))

        # ---------------- persistent tiles ----------------
        xn = [left.tile([128, N], bf16, name=f"xn{t}", tag=f"xn{t}") for t in range(T)]
        ksb = [left.tile([128, N], bf16, name=f"ksb{t}", tag=f"ksb{t}") for t in range(T)]
        qsb = [left.tile([128, NQ], bf16, name=f"qsb{t}", tag=f"qsb{t}") for t in range(T)]
        wTq = [left.tile([128, 1536], bf16, name=f"wTq{t}", tag=f"wTq{t}") for t in range(T)]
        wTp = [left.tile([128, C], bf16, name=f"wTp{t}", tag=f"wTp{t}") for t in range(T)]
        vb_bc = left.tile([128, C], f32, name="vb_bc", tag="vb_bc")
        ones_row = left.tile([1, D], f32, name="ones_row", tag="ones_row")
        qb = [left.tile([128, 1], f32, name=f"qb{i}", tag=f"qb{i}") for i in range(8)]
        pb = [left.tile([128, 1], f32, name=f"pb{i}", tag=f"pb{i}") for i in range(T)]
        nwt = [left.tile([128, 1], f32, name=f"nwt{t}", tag=f"nwt{t}") for t in range(T)]
        nbt = [left.tile([128, 1], f32, name=f"nbt{t}", tag=f"nbt{t}") for t in range(T)]
        stat = [left.tile([128, 2], f32, name=f"stat{t}", tag=f"stat{t}") for t in range(T)]
        gstat = [left.tile([128, 2], f32, name=f"gstat{t}", tag=f"gstat{t}") for t in range(T)]

        # ---------------- input DMAs ----------------
        xf = [xf_pool.tile([128, N], f32, name=f"xf{t}", tag=f"xf{t}") for t in range(T)]
        for t in range(T):
            for c4 in range(4):   # split across DMA queues
                nc.sync.dma_start(
                    xf[t][:, 1024 * c4:1024 * (c4 + 1)],
                    xb[128 * t:128 * (t + 1), 1024 * c4:1024 * (c4 + 1)])
            nc.sync.dma_start(nwt[t][:, 0:1], nw[128 * t:128 * (t + 1)])
            nc.sync.dma_start(nbt[t][:, 0:1], nb[128 * t:128 * (t + 1)])
            nc.sync.dma_start(pb[t][:, 0:1], projb[128 * t:128 * (t + 1)])
        for i in range(8):
            nc.sync.dma_start(qb[i][:, 0:1], qkvb[128 * i:128 * (i + 1)])
        # v bias broadcast to 128 partitions (stride-0 partition read)
        nc.gpsimd.dma_start(
            out=vb_bc[:],
            in_=bass.AP(tensor=qkvb.tensor, offset=1024, ap=[[0, 128], [1, C]]))
        nc.vector.memset(ones_row[0:1, :], 1.0)

        # weights: natural-layout contiguous DMA, cast to bf16, transpose
        # 128x128 blocks on the PE (identity trick) into wTq/wTp.
        # identity + group-indicator matrices come in as constant inputs
        # (gpsimd ucode for iota/affine_select is unavailable here)
        ident = left.tile([128, 128], bf16, name="ident", tag="ident")
        nc.sync.dma_start(ident[:], io["cid"][:, :])
        ind = left.tile([128, 2], f32, name="ind", tag="ind")
        nc.sync.dma_start(ind[:], io["cind"][:, :])
        indT = left.tile([2, 128], f32, name="indT", tag="indT")
        nc.sync.dma_start(indT[0:2, :], io["cindT"][:, :])
        for i in range(12):   # qkv_w row-tiles
            wstg = wstg_pool.tile([128, C], f32, name="wstg", tag="wstg")
            nc.sync.dma_start(wstg[:], qkvw[128 * i:128 * (i + 1), :])
            wbf = wstg_pool.tile([128, C], bf16, name="wbf", tag="wbf")
            nc.vector.tensor_copy(wbf[:], wstg[:])
            for j in range(T):
                tp = psum_big.tile([128, 128], bf16, name="tp", tag="sbig")
                nc.tensor.transpose(tp[:], wbf[:, 128 * j:128 * (j + 1)], ident[:])
                nc.vector.tensor_copy(wTq[j][:, 128 * i:128 * (i + 1)], tp[:])
        for i in range(4):    # proj_w row-tiles
            wstg = wstg_pool.tile([128, C], f32, name="wstg", tag="wstg")
            nc.sync.dma_start(wstg[:], projw[128 * i:128 * (i + 1), :])
            wbf = wstg_pool.tile([128, C], bf16, name="wbf", tag="wbf")
            nc.vector.tensor_copy(wbf[:], wstg[:])
            for j in range(T):
                tp = psum_big.tile([128, 128], bf16, name="tp", tag="sbig")
                nc.tensor.transpose(tp[:], wbf[:, 128 * j:128 * (j + 1)], ident[:])
                nc.vector.tensor_copy(wTp[j][:, 128 * i:128 * (i + 1)], tp[:])

        # ---------------- phase 1: group stats ----------------
        for t in range(T):
            nc.vector.tensor_reduce(
                out=stat[t][:, 0:1], in_=xf[t][:], axis=mybir.AxisListType.X, op=Alu.add)
            sq_scr = scr_pool.tile([128, N], bf16, name="sq_scr", tag="sq_scr")
            nc.scalar.activation(
                sq_scr[:], xf[t][:], Act.Square, accum_out=stat[t][:, 1:2])
            # group-reduce over partitions via indicator matmuls:
            # gg[g,s] = sum_ch ind[ch,g]*stat[ch,s]; then broadcast back
            # per channel: gstat[ch,s] = sum_g indT[g,ch]*gg[g,s]
            gg_ps = psum_acc.tile([2, 2], f32, name="gg_ps", tag="acc")
            nc.tensor.matmul(gg_ps[0:2, :], ind[:, 0:2], stat[t][:, 0:2],
                             start=True, stop=True)
            gg_sb = left.tile([2, 2], f32, name=f"gg_sb{t}", tag=f"gg_sb{t}")
            nc.vector.tensor_copy(gg_sb[0:2, :], gg_ps[0:2, :])
            gb_ps = psum_acc.tile([128, 2], f32, name="gb_ps", tag="acc")
            nc.tensor.matmul(gb_ps[:, 0:2], indT[0:2, :], gg_sb[0:2, :],
                             start=True, stop=True)
            nc.vector.tensor_copy(gstat[t][:, 0:2], gb_ps[:, 0:2])
            # mean/var/rstd -> per-channel affine a,b
            mean_t = left.tile([128, 1], f32, name=f"mean{t}", tag=f"mean{t}")
            e2_t = left.tile([128, 1], f32, name=f"e2{t}", tag=f"e2{t}")
            var_t = left.tile([128, 1], f32, name=f"var{t}", tag=f"var{t}")
            std_t = left.tile([128, 1], f32, name=f"std{t}", tag=f"std{t}")
            a_t = left.tile([128, 1], f32, name=f"a{t}", tag=f"a{t}")
            b_t = left.tile([128, 1], f32, name=f"b{t}", tag=f"b{t}")
            inv = 1.0 / GELEM
            nc.vector.tensor_scalar(mean_t[:], gstat[t][:, 0:1], inv, None, Alu.mult)
            nc.vector.tensor_scalar(e2_t[:], gstat[t][:, 1:2], inv, None, Alu.mult)
            nc.vector.scalar_tensor_tensor(
                var_t[:], mean_t[:], -1.0, mean_t[:], Alu.mult, Alu.mult)
            nc.vector.scalar_tensor_tensor(
                var_t[:], e2_t[:], EPS, var_t[:], Alu.add, Alu.add)
            nc.scalar.activation(std_t[:], var_t[:], Act.Sqrt)
            nc.vector.reciprocal(a_t[:], std_t[:])
            nc.vector.tensor_tensor(a_t[:], a_t[:], nwt[t][:], Alu.mult)
            nc.vector.tensor_tensor(b_t[:], mean_t[:], a_t[:], Alu.mult)
            nc.vector.tensor_tensor(b_t[:], nbt[t][:], b_t[:], Alu.subtract)
            # phase 2: normalize + cast
            nc.vector.tensor_scalar(
                xn[t][:], xf[t][:], a_t[:, 0:1], b_t[:, 0:1], Alu.mult, Alu.add)
            if DEBUG:
                d = io["dbg_ab"]
                nc.sync.dma_start(d[128 * t:128 * t + 128, 0:1], a_t[:])
                nc.sync.dma_start(d[128 * t:128 * t + 128, 1:2], b_t[:])
                nc.sync.dma_start(d[128 * t:128 * t + 128, 2:3], stat[t][:, 0:1])
                nc.sync.dma_start(d[128 * t:128 * t + 128, 3:4], stat[t][:, 1:2])
                nc.sync.dma_start(d[128 * t:128 * t + 128, 4:5], gstat[t][:, 0:1])
                nc.sync.dma_start(d[128 * t:128 * t + 128, 5:6], gstat[t][:, 1:2])
                nc.sync.dma_start(io["dbg_xn"][128 * t:128 * t + 128, :], xn[t][:, 0:64])

        right_ctx.close()

        # ---------------- mid pools (reuse xf space) ----------------
        mid = ctx.enter_context(tc.tile_pool(name="mid", bufs=1))
        psb_pool = ctx.enter_context(tc.tile_pool(name="psb_pool", bufs=4))
        rec_pool = ctx.enter_context(tc.tile_pool(name="rec_pool", bufs=2))
        yo_pool = ctx.enter_context(tc.tile_pool(name="yo_pool", bufs=2))

        vT = mid.tile([128, MT * 520], bf16, name="vT", tag="vT")
        attn = [mid.tile([128, NQ], bf16, name=f"attn{t}", tag=f"attn{t}") for t in range(T)]
        xres = [mid.tile([128, NQ], f32, name=f"xres{t}", tag=f"xres{t}") for t in range(T)]
        for t in range(T):
            nc.sync.dma_start(xres[t][:], xb[128 * t:128 * (t + 1), 0:NQ])

        # ones columns of the augmented v^T (denominator trick)
        ones_view = vT[:].rearrange("p (m h x) -> p m h x", m=MT, x=65)[:, :, :, 64:65]
        nc.vector.memset(ones_view, 1.0)

        # ---------------- phase 3: projections ----------------
        # q: out rows 0..511 of qkv, only NQ query columns
        for i in range(T):
            for w in range(W):
                qp = psum_big.tile([128, 512], f32, name="qp", tag="sbig")
                for k in range(T):
                    nc.tensor.matmul(
                        qp[:], wTq[k][:, 128 * i:128 * i + 128],
                        xn[k][:, 512 * w:512 * w + 512],
                        start=(k == 0), stop=(k == T - 1))
                nc.vector.tensor_scalar(
                    qsb[i][:, 512 * w:512 * w + 512], qp[:], qb[i][:, 0:1], None, Alu.add)
        # k: out rows 512..1023, all N columns
        for i in range(T):
            for w in range(N // 512):
                kp = psum_big.tile([128, 512], f32, name="kp", tag="sbig")
                for k in range(T):
                    nc.tensor.matmul(
                        kp[:], wTq[k][:, 512 + 128 * i:512 + 128 * i + 128],
                        xn[k][:, 512 * w:512 * w + 512],
                        start=(k == 0), stop=(k == T - 1))
                nc.vector.tensor_scalar(
                    ksb[i][:, 512 * w:512 * w + 512], kp[:], qb[4 + i][:, 0:1], None, Alu.add)
        # vT: (m, 512) per m-tile, strided into the 65-column augmented layout
        for mt in range(MT):
            vp = psum_big.tile([128, 512], f32, name="vp", tag="sbig")
            for k in range(T):
                nc.tensor.matmul(
                    vp[:], xn[k][:, 128 * mt:128 * mt + 128],
                    wTq[k][:, 1024:1536],
                    start=(k == 0), stop=(k == T - 1))
            dst = vT[:, 520 * mt:520 * mt + 520].rearrange(
                "p (h x) -> p h x", x=65)[:, :, 0:64]
            src = vp[:].rearrange("p (h x) -> p h x", x=64)
            vbv = vb_bc[:].rearrange("p (h x) -> p h x", x=64)
            nc.vector.tensor_tensor(dst, src, vbv, Alu.add)
            if DEBUG and mt == 0:
                nc.sync.dma_start(io["dbg_vt"][:], vT[:, 0:520])
                nc.sync.dma_start(io["dbg_k"][0:128, :], ksb[0][:, 0:64])
                nc.sync.dma_start(io["dbg_q"][0:128, :], qsb[0][:, 0:64])

        # ---------------- phase 4: attention ----------------
        for w in range(W):
            for h in range(H):
                kt, pr = h // 2, 64 * (h % 2)
                pv = psum_pv.tile([128, 512], f32, name="pv", tag="pv")
                mt = 0
                for gs in MG:
                    sp = psum_big.tile([128, 1536], f32, name="sp", tag="sbig")
                    for j in range(gs):
                        nc.tensor.matmul(
                            sp[:, 512 * j:512 * j + 512],
                            ksb[kt][pr:pr + 64, 128 * (mt + j):128 * (mt + j) + 128],
                            qsb[kt][pr:pr + 64, 512 * w:512 * w + 512],
                            start=True, stop=True)
                    ps = psb_pool.tile([128, 1536], bf16, name="ps", tag="ps")
                    nc.scalar.activation(
                        ps[:, 0:512 * gs], sp[:, 0:512 * gs], Act.Exp, scale=0.125)
                    for j in range(gs):
                        m = mt + j
                        nc.tensor.matmul(
                            pv[0:65, :],
                            vT[:, 520 * m + 65 * h:520 * m + 65 * h + 65],
                            ps[:, 512 * j:512 * j + 512],
                            start=(m == 0), stop=(m == MT - 1))
                    mt += gs
                # NOTE: reciprocal_approx_* mis-handles nonzero partition
                # offsets on HW (reads partition 0), so stage the denominator
                # row at partition 0 first
                dnm = rec_pool.tile([1, 512], f32, name="dnm", tag="dnm")
                nc.vector.tensor_copy(dnm[0:1, :], pv[64:65, :])
                if DEBUG and w == 0:
                    dd2 = rec_pool.tile([1, 512], f32, name="dd2", tag="dd2")
                    nc.vector.tensor_copy(dd2[0:1, :], pv[0:1, :])
                    nc.sync.dma_start(io["dbg_den"][h:h + 1, :], dnm[0:1, :])
                    nc.sync.dma_start(io["dbg_pv"][h:h + 1, :], dd2[0:1, :])
                rec = rec_pool.tile([1, 512], f32, name="rec", tag="rec")
                rscr = rec_pool.tile([1, 512], f32, name="rscr", tag="rscr")
                nc.vector.reciprocal_approx_accurate(
                    rec[0:1, :], dnm[0:1, :], rscr[0:1, :])
                bc = psum_acc.tile([128, 512], f32, name="bc", tag="acc")
                nc.tensor.matmul(
                    bc[0:64, :], ones_row[0:1, 0:D],
                    rec[0:1, :], start=True, stop=True)
                bcs = rec_pool.tile([64, 512], f32, name="bcs", tag="bcs")
                nc.vector.tensor_copy(bcs[0:64, :], bc[0:64, :])
                nc.vector.tensor_tensor(
                    attn[kt][pr:pr + 64, 512 * w:512 * w + 512],
                    pv[0:64, :], bcs[0:64, :], Alu.mult)

            # ---------------- phase 5: proj + residual for this window ----
            for i in range(T):
                py = psum_acc.tile([128, 512], f32, name="py", tag="acc")
                # shares the 1-bank acc pool with bc; proj overlaps attention
                # of the next window only through this slot
                for k in range(T):
                    nc.tensor.matmul(
                        py[:], wTp[k][:, 128 * i:128 * i + 128],
                        attn[k][:, 512 * w:512 * w + 512],
                        start=(k == 0), stop=(k == T - 1))
                yo = yo_pool.tile([128, 512], f32, name="yo", tag="yo")
                nc.vector.scalar_tensor_tensor(
                    yo[:], py[:], pb[i][:, 0:1], xres[i][:, 512 * w:512 * w + 512],
                    Alu.add, Alu.add)
                nc.sync.dma_start(y[128 * i:128 * i + 128, 512 * w:512 * w + 512], yo[:])


def _build():
    import concourse.tile as tile
    from concourse import bacc, mybir

    nc = bacc.Bacc("TRN2", target_bir_lowering=False, debug=False)
    f32 = mybir.dt.float32
    io = {
        "xb": nc.dram_tensor("xb", [C, N], f32, kind="ExternalInput").ap(),
        "qkvw": nc.dram_tensor("qkvw", [3 * C, C], f32, kind="ExternalInput").ap(),
        "qkvb": nc.dram_tensor("qkvb", [3 * C], f32, kind="ExternalInput").ap(),
        "projw": nc.dram_tensor("projw", [C, C], f32, kind="ExternalInput").ap(),
        "projb": nc.dram_tensor("projb", [C], f32, kind="ExternalInput").ap(),
        "nw": nc.dram_tensor("nw", [C], f32, kind="ExternalInput").ap(),
        "nb": nc.dram_tensor("nb", [C], f32, kind="ExternalInput").ap(),
        "cid": nc.dram_tensor("cid", [128, 128], mybir.dt.bfloat16,
                              kind="ExternalInput").ap(),
        "cind": nc.dram_tensor("cind", [128, 2], f32, kind="ExternalInput").ap(),
        "cindT": nc.dram_tensor("cindT", [2, 128], f32, kind="ExternalInput").ap(),
        "y": nc.dram_tensor("y", [C, NQ], f32, kind="ExternalOutput").ap(),
    }
    if DEBUG:
        bf16 = mybir.dt.bfloat16
        io["dbg_ab"] = nc.dram_tensor("dbg_ab", [C, 8], f32, kind="ExternalOutput").ap()
        io["dbg_xn"] = nc.dram_tensor("dbg_xn", [C, 64], bf16, kind="ExternalOutput").ap()
        io["dbg_vt"] = nc.dram_tensor("dbg_vt", [128, 520], bf16, kind="ExternalOutput").ap()
        io["dbg_k"] = nc.dram_tensor("dbg_k", [C, 64], bf16, kind="ExternalOutput").ap()
        io["dbg_q"] = nc.dram_tensor("dbg_q", [C, 64], bf16, kind="ExternalOutput").ap()
        io["dbg_den"] = nc.dram_tensor("dbg_den", [8, 512], f32, kind="ExternalOutput").ap()
        io["dbg_pv"] = nc.dram_tensor("dbg_pv", [8, 512], f32, kind="ExternalOutput").ap()
    with tile.TileContext(nc) as tc:
        _emit(tc, io)
    nc.compile()
    return nc


def get_compiled():
    global _COMPILED
    if _COMPILED is None:
        _COMPILED = _build()
    return _COMPILED


def make_in_maps(x, norm_w, norm_b, qkv_w, qkv_b, proj_w, proj_b):
    import ml_dtypes

    xf = np.ascontiguousarray(np.asarray(x, np.float32)).reshape(2, C, N)
    ind = np.zeros((128, 2), np.float32)
    ind[0:64, 0] = 1.0
    ind[64:128, 1] = 1.0
    shared = {
        "cid": np.eye(128, dtype=ml_dtypes.bfloat16),
        "cind": ind,
        "cindT": np.ascontiguousarray(ind.T),
        "qkvw": np.ascontiguousarray(np.asarray(qkv_w, np.float32)),
        "qkvb": np.ascontiguousarray(np.asarray(qkv_b, np.float32)),
        "projw": np.ascontiguousarray(np.asarray(proj_w, np.float32)),
        "projb": np.ascontiguousarray(np.asarray(proj_b, np.float32)),
        "nw": np.ascontiguousarray(np.asarray(norm_w, np.float32)),
        "nb": np.ascontiguousarray(np.asarray(norm_b, np.float32)),
    }
    in_maps = []
    for core in range(8):
        bi, qs = core // 4, core % 4
        # rotate so this core's queries are always columns [0:NQ)
        xroll = np.concatenate(
            [xf[bi][:, qs * NQ:], xf[bi][:, :qs * NQ]], axis=1)
        m = dict(shared)
        m["xb"] = np.ascontiguousarray(xroll)
        in_maps.append(m)
    return in_maps


def assemble(results, x):
    y = np.zeros((2, C, N), np.float32)
    for core in range(8):
        bi, qs = core // 4, core % 4
        y[bi][:, qs * NQ:(qs + 1) * NQ] = results[core]["y"]
    return y.reshape(x.shape)


def kernel(x, norm_w, norm_b, qkv_w, qkv_b, proj_w, proj_b, **_ignored):
    from concourse import bass_utils

    nc = get_compiled()
    in_maps = make_in_maps(x, norm_w, norm_b, qkv_w, qkv_b, proj_w, proj_b)
    res = bass_utils.run_bass_kernel_spmd(nc, in_maps, core_ids=list(range(8)))
    return assemble(res.results, np.asarray(x))
